# revision 43
# baseline (speedup 1.0000x reference)
"""Single-head causal attention (B=8, T=2048, C=1024, H=64) on 8 TRN2 NeuronCores.

Sharding: data-parallel over batch -- core b computes batch element b. No
collectives. Per core, for x_b [T, C]:
    q = x_b @ Wq / sqrt(H); k = x_b @ Wk; v = x_b @ Wv
    out = softmax(causal(q @ k.T)) @ v

v3 layout strategy (all PE work in bf16, 1 cycle/row + FWL weight loads):
  - Host passes xT = x_b.T [C, T] pre-converted to bf16 (halves input DMA)
    and wqkv packed [128, 8, 192] bf16 (Wq/sqrt(H) | Wk | Wv per C-chunk) so
    weights land in one contiguous DMA (chunk 0 shipped first).
  - Projections per 512-wide t-group: accumulate QK^T [128, 512] and V^T
    [64, 512] in PSUM over 8 C-chunks. Q^T and K^T are copied out REPLICATED
    on both partition halves (qq/kk [128, T]) so the two scores matmuls of a
    k-tile pair run CONCURRENTLY in disjoint 64-row groups of the PE array
    (contraction is only 64 deep). V^T is re-tiled pairwise into vt2
    [128, 8, 128]; ONE PE transpose per pair of 128-k-tiles yields natural V
    [128k, 2x64h]. V tiles live padded in vsb [128, 16, 128]: col 64 = ones
    (softmax denominators fall out of the O matmul), cols 65:128 = zero.
  - Attention per 512-wide q-group g over k-tile pairs: two concurrent
    full-width scores matmuls S^T = K_j^T.T @ Q^T into a 2-bank PSUM tile,
    ONE exp over [128, 1024] on ACT (no max subtraction; scores bounded
    ~+-7), causal 0/1 mask multiply on diagonal 128x128 sub-blocks, then
    per-tile O^T[128, qlo:512] += Vpad_j.T @ P^T accumulated in PSUM (row 64
    = softmax denominator). The O matmuls of a pair are issued one pair
    BEHIND the scores matmuls, and the next group's projection matmuls are
    interleaved into the attention stream, so the PE never drains waiting
    for exp.
  - Normalize: ACT copy of the denominator row to partition 0, DVE
    reciprocal_approx_fast, GPSIMD partition broadcast, DVE multiply; DMA
    O^T [64, T] f32 out. Host transposes back.
"""

from contextlib import ExitStack

import ml_dtypes
import numpy as np

import concourse.mybir as mybir
import concourse.tile as tile
from concourse import bacc
from concourse.bass_utils import run_bass_kernel_spmd
from concourse.masks import make_identity, make_upper_triangular

B, T, C, H = 8, 2048, 1024, 64
N_CORES = 8
GQ = 512          # q-group width (PSUM bank)
NG = T // GQ      # 4 q-groups
KT = 128          # k-tile size
CC = C // 128     # 8 contraction chunks
F32 = mybir.dt.float32
BF = mybir.dt.bfloat16
EXP = mybir.ActivationFunctionType.Exp


class _State:
    pass


def _emit(ctx, tc):
    nc = tc.nc
    st = _State()
    st.nc = nc
    st.xT = nc.dram_tensor("xT", [C, T], BF, kind="ExternalInput").ap()
    wqkv = nc.dram_tensor("wqkv", [128, CC, 3 * H], BF, kind="ExternalInput").ap()
    st.outT = nc.dram_tensor("outT", [H, T], F32, kind="ExternalOutput").ap()

    const = ctx.enter_context(tc.tile_pool(name="const", bufs=1))
    persist = ctx.enter_context(tc.tile_pool(name="persist", bufs=1))
    st.xt_pool = ctx.enter_context(tc.tile_pool(name="xt", bufs=4))
    st.pt_pool = ctx.enter_context(tc.tile_pool(name="pt", bufs=4))
    st.out_pool = ctx.enter_context(tc.tile_pool(name="outp", bufs=2))
    st.div_pool = ctx.enter_context(tc.tile_pool(name="div", bufs=2))
    st.ps_s = ctx.enter_context(tc.tile_pool(name="ps_s", bufs=2, space="PSUM"))
    st.ps_o = ctx.enter_context(tc.tile_pool(name="ps_o", bufs=2, space="PSUM"))
    st.ps_qk = ctx.enter_context(tc.tile_pool(name="ps_qk", bufs=1, space="PSUM"))
    st.ps_vtr = ctx.enter_context(tc.tile_pool(name="ps_vtr", bufs=1, space="PSUM"))

    st.wsb = const.tile([128, CC, 3 * H], BF)
    st.x0 = persist.tile([128, CC, GQ], BF)     # prologue x tiles (group 0)
    # first compute needs x0 chunk 0 + wsb chunk 0: trigger those DMAs first
    nc.sync.dma_start(out=st.x0[:, 0:1, :],
                      in_=xT_first(st, 0, 1))
    nc.sync.dma_start(out=st.wsb[:, 0:2, :], in_=wqkv[:, 0:2, :])
    nc.sync.dma_start(out=st.x0[:, 1:4, :],
                      in_=xT_first(st, 1, 4))
    nc.sync.dma_start(out=st.wsb[:, 2:CC, :], in_=wqkv[:, 2:CC, :])
    nc.sync.dma_start(out=st.x0[:, 4:CC, :],
                      in_=xT_first(st, 4, CC))
    # mask[p, f] = 1.0 where p <= f else 0 : keep k_local <= q_local.
    maskf = const.tile([128, 128], F32)
    make_upper_triangular(nc, maskf[:], val=1.0, diag=True)
    st.mask = const.tile([128, 128], BF)
    nc.scalar.copy(st.mask[:], maskf[:])
    identf = const.tile([128, 128], F32)
    make_identity(nc, identf[:])
    st.ident = const.tile([128, 128], BF)
    nc.scalar.copy(st.ident[:], identf[:])

    st.qt = persist.tile([H, T], BF)            # Q^T (pre-scaled by 1/sqrt(H))
    st.kt = persist.tile([H, T], BF)            # K^T
    st.vt2 = persist.tile([128, T // (2 * KT), 128], BF)  # V^T pair-packed
    st.vsb = persist.tile([128, T // KT, 128], BF)  # natural V tiles, padded
    # vsb column layout: [ones | zero pad x63 | V x64] so the O matmul puts
    # the softmax denominator on PSUM partition 0 and O^T on 64:128
    nc.vector.memset(st.vsb[:, :, 0:1], 1.0)
    nc.vector.memset(st.vsb[:, :, 1:H], 0.0)

    # preload the ACT exp table (~1.3us) during the DMA-wait head
    dummy = const.tile([1, 2], F32)
    nc.scalar.activation(dummy[:, 0:1], maskf[0:1, 0:1], EXP)

    # PE warmup: ~4us of back-to-back dummy matmuls while the first DMAs
    # land, so the tensor engine reaches its max p-state before real work
    warm = st.ps_s.tile([128, 2, GQ], F32, name="s_ps")
    for _ in range(36):
        nc.tensor.matmul(warm[:, 0, 0:128], st.ident[:], st.ident[:],
                         start=True, stop=True)

    # ---- flat cross-group software-pipelined schedule ----
    # The O matmuls run one pair behind the scores matmuls, ACROSS group
    # boundaries, so the PE always has independent work queued while an exp
    # completes. Next-group projection chunks ride the front half of each
    # group's pairs (their PSUM->SBUF copies land early in the ACT/DVE
    # queues); V transposes ride pair 1 of their own group.
    _proj_prologue(st)
    _proj_tr(st, 0)
    chunk_at, copies_at, tr_at = {}, {}, {}
    for g in range(NG - 1):
        front = max(1, (2 * g + 2) // 2)
        for i in range(CC):
            chunk_at.setdefault((g, i * front // CC), []).append(i)
        copies_at[(g, front - 1)] = g + 1
        # transposes ride the LAST pair of group g: vtr(g+1) accumulation and
        # copies are done by then, and vtr(g+2) isn't created until (g+1, 0)
        tr_at[(g, 2 * g + 1)] = g + 1
    o_ps_of = {}
    prev = None
    for g in range(NG):
        sl = slice(GQ * g, GQ * (g + 1))
        for p in range(2 * g + 2):
            if p == 0:
                o_ps_of[g] = (st.ps_o.tile([128, GQ], F32, name="o_ps"),)
            cur = (g, p, sl, _pair_scores(st, g, p, sl))
            if prev is not None:
                pg, pp, psl, ppt = prev
                _pair_finish(st, pg, pp, psl, ppt, o_ps_of[pg])
                if pp == 2 * pg + 1:
                    _normalize(st, pg, psl, o_ps_of[pg][0], 0, GQ)
            _proj_chunks(st, g + 1, chunk_at.get((g, p), []))
            if (g, p) in copies_at:
                _proj_copies(st, copies_at[(g, p)])
            if (g, p) in tr_at:
                _proj_tr(st, tr_at[(g, p)])
            prev = cur
    pg, pp, psl, ppt = prev
    _pair_finish(st, pg, pp, psl, ppt, o_ps_of[pg])
    _normalize(st, pg, psl, o_ps_of[pg][0], 0, GQ)


def xT_first(st, lo, hi):
    return st.xT[128 * lo : 128 * hi, 0:GQ].rearrange("(a p) t -> p a t", p=128)


def _proj_prologue(st):
    """Group 0 projection (x0 DMAs already issued up top): all qk matmuls
    before the v matmuls so the Q/K copy-out (which gates the first scores
    matmul) overlaps the v matmuls on the PE."""
    nc = st.nc
    st.qk_ps = st.ps_qk.tile([128, GQ], F32)
    st.vtr = st.ps_vtr.tile([128, GQ], F32)
    for ci in range(CC):
        nc.tensor.matmul(st.qk_ps[:], st.wsb[:, ci, 0:128], st.x0[:, ci, :],
                         start=(ci == 0), stop=(ci == CC - 1))
    _proj_copies_qk(st, 0)
    for ci in range(CC):
        nc.tensor.matmul(st.vtr[0:H, :], st.wsb[:, ci, 128:192],
                         st.x0[:, ci, :],
                         start=(ci == 0), stop=(ci == CC - 1))
    _proj_copies_v(st, 0)


def _proj_chunks(st, g, cis):
    nc = st.nc
    sl = slice(GQ * g, GQ * (g + 1))
    for ci in cis:
        if ci == 0:
            st.qk_ps = st.ps_qk.tile([128, GQ], F32)
            st.vtr = st.ps_vtr.tile([128, GQ], F32)
        if ci % 4 == 0:
            # one batched DMA per 4 C-chunks (4 KB/partition lines)
            st.xt_t = st.xt_pool.tile([128, 4, GQ], BF)
            nc.sync.dma_start(
                out=st.xt_t[:],
                in_=st.xT[512 * (ci // 4) : 512 * (ci // 4 + 1), sl].rearrange(
                    "(a p) t -> p a t", p=128))
        xt = st.xt_t[:, ci % 4, :]
        nc.tensor.matmul(st.qk_ps[:], st.wsb[:, ci, 0:128], xt,
                         start=(ci == 0), stop=(ci == CC - 1))
        nc.tensor.matmul(st.vtr[0:H, :], st.wsb[:, ci, 128:192], xt,
                         start=(ci == 0), stop=(ci == CC - 1))


def _proj_copies_qk(st, g):
    nc = st.nc
    sl = slice(GQ * g, GQ * (g + 1))
    nc.vector.tensor_copy(st.kt[:, sl], st.qk_ps[H:128, :])  # shifted, DVE
    nc.vector.tensor_copy(st.qt[:, sl], st.qk_ps[0:H, :])


def _proj_copies_v(st, g):
    # V^T -> pair-packed vt2
    nc = st.nc
    for e in range(2):
        u = 2 * g + e
        nc.vector.tensor_copy(st.vt2[0:H, u, :],
                              st.vtr[0:H, 256 * e : 256 * e + 128])
        nc.scalar.copy(st.vt2[H:128, u, :],      # shifted -> ACT
                       st.vtr[0:H, 256 * e + 128 : 256 * e + 256])


def _proj_copies(st, g):
    _proj_copies_qk(st, g)
    _proj_copies_v(st, g)


def _proj_tr(st, g):
    # PE-transpose vt2 pairs to natural V tiles (fills the group-end drain)
    nc = st.nc
    vtr_b = st.vtr.bitcast(BF)                  # reuse the V PSUM bank
    for e in range(2):
        u = 2 * g + e
        nc.tensor.transpose(vtr_b[:, 128 * e : 128 * (e + 1)],
                            st.vt2[:, u, :], st.ident[:])
        nc.vector.tensor_copy(st.vsb[:, 2 * u, H:128],
                              vtr_b[:, 128 * e : 128 * e + H])
        nc.vector.tensor_copy(st.vsb[:, 2 * u + 1, H:128],
                              vtr_b[:, 128 * e + H : 128 * e + 128])


def _pair_scores(st, g, p, sl):
    # columns below the pair's min qlo are fully masked for BOTH tiles of
    # the pair -- skip them in the scores matmuls and the exp
    nc = st.nc
    lo = max(0, 128 * (2 * p - 4 * g))
    s_ps = st.ps_s.tile([128, 2, GQ], F32)
    for e in range(2):
        j = 2 * p + e
        nc.tensor.matmul(s_ps[:, e, lo:GQ], st.kt[:, KT * j : KT * (j + 1)],
                         st.qt[:, GQ * g + lo : GQ * (g + 1)],
                         start=True, stop=True)
    pt_t = st.pt_pool.tile([128, 2, GQ], BF)
    nc.scalar.activation(pt_t[:, :, lo:GQ], s_ps[:, :, lo:GQ], EXP)
    return pt_t


def _pair_finish(st, g, p, sl, pt_t, o_ps):
    # o_ps: tuple of 1 (single-bank) or 2 (column-split, last group) tiles
    nc = st.nc
    jmax = 4 * g + 3
    for e in range(2):
        j = 2 * p + e
        s = j - 4 * g                           # diagonal sub-block index
        if s >= 0:
            qlo = 128 * s
            nc.gpsimd.tensor_mul(pt_t[:, e, qlo : qlo + 128],
                                 pt_t[:, e, qlo : qlo + 128], st.mask[:])
    for e in range(2):
        j = 2 * p + e
        qlo = max(0, 128 * (j - 4 * g))
        if len(o_ps) == 1:
            nc.tensor.matmul(o_ps[0][:, qlo:GQ], st.vsb[:, j, :],
                             pt_t[:, e, qlo:GQ],
                             start=(j == 0), stop=(j == jmax))
        else:
            oa, ob = o_ps
            if qlo < 256:
                nc.tensor.matmul(oa[:, qlo:256], st.vsb[:, j, :],
                                 pt_t[:, e, qlo:256],
                                 start=(j == 0), stop=(j == 4 * g + 1))
            lob = max(qlo, 256)
            nc.tensor.matmul(ob[:, lob:GQ], st.vsb[:, j, :],
                             pt_t[:, e, lob:GQ],
                             start=(j == 0), stop=(j == jmax))


def _normalize(st, g, sl, o_ps, lo, hi):
    nc = st.nc
    rec = st.div_pool.tile([1, GQ], F32, name="rec")
    nc.vector.reciprocal_approx_fast(rec[:, lo:hi], o_ps[0:1, lo:hi])
    dbc = st.div_pool.tile([128, GQ], F32, name="dbc")
    nc.gpsimd.partition_broadcast(dbc[:, lo:hi], rec[:, lo:hi])
    osb = st.out_pool.tile([128, GQ], F32, name="osb")
    nc.vector.tensor_mul(osb[H:128, lo:hi], o_ps[H:128, lo:hi],
                         dbc[H:128, lo:hi])
    nc.sync.dma_start(out=st.outT[:, GQ * g + lo : GQ * g + hi],
                      in_=osb[H:128, lo:hi])


def build():
    nc = bacc.Bacc("TRN2", target_bir_lowering=False, debug=False)
    with tile.TileContext(nc) as tc:
        with ExitStack() as ctx:
            _emit(ctx, tc)
    nc.compile()
    return nc


_NC_CACHE = None


def _get_module():
    global _NC_CACHE
    if _NC_CACHE is None:
        _NC_CACHE = build()
    return _NC_CACHE


def prep_in_maps(x, Wq, Wk, Wv):
    x = np.asarray(x, dtype=np.float32)
    Wq = np.asarray(Wq, dtype=np.float32)
    Wk = np.asarray(Wk, dtype=np.float32)
    Wv = np.asarray(Wv, dtype=np.float32)
    bf16 = ml_dtypes.bfloat16
    # [C, 192] = [Wq/sqrt(H) | Wk | Wv], tiled to [128, CC, 192]
    wcat = np.concatenate([Wq * (1.0 / np.sqrt(H)), Wk, Wv], axis=1)
    wqkv = np.ascontiguousarray(
        wcat.reshape(CC, 128, 3 * H).transpose(1, 0, 2).astype(bf16))
    return [
        {"xT": np.ascontiguousarray(x[b].T.astype(bf16)), "wqkv": wqkv}
        for b in range(B)
    ]


def assemble_out(results):
    out = np.empty((B, T, H), dtype=np.float32)
    for b in range(B):
        out[b] = results[b]["outT"].T
    return out


def run(x, Wq, Wk, Wv, trace=False):
    nc = _get_module()
    in_maps = prep_in_maps(x, Wq, Wk, Wv)
    res = run_bass_kernel_spmd(nc, in_maps, core_ids=list(range(N_CORES)),
                               trace=trace)
    return assemble_out(res.results), res


def kernel(x, Wq, Wk, Wv):
    out, _ = run(x, Wq, Wk, Wv)
    return out


# revision 44
# speedup vs baseline: 1.4309x; 1.4309x over previous
"""Single-head causal attention (B=8, T=2048, C=1024, H=64) on 8 TRN2 NeuronCores.

Sharding: data-parallel over batch -- core b computes batch element b. No
collectives. Per core, for x_b [T, C]:
    q = x_b @ Wq / sqrt(H); k = x_b @ Wk; v = x_b @ Wv
    out = softmax(causal(q @ k.T)) @ v

v3 layout strategy (all PE work in bf16, 1 cycle/row + FWL weight loads):
  - Host passes xT = x_b.T [C, T] pre-converted to bf16 (halves input DMA)
    and wqkv packed [128, 8, 192] bf16 (Wq/sqrt(H) | Wk | Wv per C-chunk) so
    weights land in one contiguous DMA (chunk 0 shipped first).
  - Projections per 512-wide t-group: accumulate QK^T [128, 512] and V^T
    [64, 512] in PSUM over 8 C-chunks. Q^T and K^T are copied out REPLICATED
    on both partition halves (qq/kk [128, T]) so the two scores matmuls of a
    k-tile pair run CONCURRENTLY in disjoint 64-row groups of the PE array
    (contraction is only 64 deep). V^T is re-tiled pairwise into vt2
    [128, 8, 128]; ONE PE transpose per pair of 128-k-tiles yields natural V
    [128k, 2x64h]. V tiles live padded in vsb [128, 16, 128]: col 64 = ones
    (softmax denominators fall out of the O matmul), cols 65:128 = zero.
  - Attention per 512-wide q-group g over k-tile pairs: two concurrent
    full-width scores matmuls S^T = K_j^T.T @ Q^T into a 2-bank PSUM tile,
    ONE exp over [128, 1024] on ACT (no max subtraction; scores bounded
    ~+-7), causal 0/1 mask multiply on diagonal 128x128 sub-blocks, then
    per-tile O^T[128, qlo:512] += Vpad_j.T @ P^T accumulated in PSUM (row 64
    = softmax denominator). The O matmuls of a pair are issued one pair
    BEHIND the scores matmuls, and the next group's projection matmuls are
    interleaved into the attention stream, so the PE never drains waiting
    for exp.
  - Normalize: ACT copy of the denominator row to partition 0, DVE
    reciprocal_approx_fast, GPSIMD partition broadcast, DVE multiply; DMA
    O^T [64, T] f32 out. Host transposes back.
"""

from contextlib import ExitStack

import ml_dtypes
import numpy as np

import concourse.mybir as mybir
import concourse.tile as tile
from concourse import bacc
from concourse.bass_utils import run_bass_kernel_spmd
from concourse.masks import make_identity, make_upper_triangular

B, T, C, H = 8, 2048, 1024, 64
N_CORES = 8
GQ = 512          # q-group width (PSUM bank)
NG = T // GQ      # 4 q-groups
KT = 128          # k-tile size
CC = C // 128     # 8 contraction chunks
F32 = mybir.dt.float32
BF = mybir.dt.bfloat16
EXP = mybir.ActivationFunctionType.Exp


class _State:
    pass


def _emit(ctx, tc):
    nc = tc.nc
    st = _State()
    st.nc = nc
    st.xT = nc.dram_tensor("xT", [C, T], BF, kind="ExternalInput").ap()
    wqkv = nc.dram_tensor("wqkv", [128, CC, 3 * H], BF, kind="ExternalInput").ap()
    st.outT = nc.dram_tensor("outT", [H, T], F32, kind="ExternalOutput").ap()

    const = ctx.enter_context(tc.tile_pool(name="const", bufs=1))
    persist = ctx.enter_context(tc.tile_pool(name="persist", bufs=1))
    st.xt_pool = ctx.enter_context(tc.tile_pool(name="xt", bufs=4))
    st.pt_pool = ctx.enter_context(tc.tile_pool(name="pt", bufs=4))
    st.out_pool = ctx.enter_context(tc.tile_pool(name="outp", bufs=2))
    st.div_pool = ctx.enter_context(tc.tile_pool(name="div", bufs=2))
    st.ps_s = ctx.enter_context(tc.tile_pool(name="ps_s", bufs=2, space="PSUM"))
    st.ps_o = ctx.enter_context(tc.tile_pool(name="ps_o", bufs=2, space="PSUM"))
    st.ps_qk = ctx.enter_context(tc.tile_pool(name="ps_qk", bufs=1, space="PSUM"))
    st.ps_vtr = ctx.enter_context(tc.tile_pool(name="ps_vtr", bufs=1, space="PSUM"))

    st.wsb = const.tile([128, CC, 3 * H], BF)
    st.x0 = persist.tile([128, CC, GQ], BF)     # prologue x tiles (group 0)
    # first compute needs x0 chunk 0 + wsb chunk 0: trigger those DMAs first
    nc.sync.dma_start(out=st.x0[:, 0:1, :],
                      in_=xT_first(st, 0, 1))
    nc.sync.dma_start(out=st.wsb[:, 0:2, :], in_=wqkv[:, 0:2, :])
    nc.sync.dma_start(out=st.x0[:, 1:4, :],
                      in_=xT_first(st, 1, 4))
    nc.sync.dma_start(out=st.wsb[:, 2:CC, :], in_=wqkv[:, 2:CC, :])
    nc.sync.dma_start(out=st.x0[:, 4:CC, :],
                      in_=xT_first(st, 4, CC))
    # mask[p, f] = 1.0 where p <= f else 0 : keep k_local <= q_local.
    maskf = const.tile([128, 128], F32)
    make_upper_triangular(nc, maskf[:], val=1.0, diag=True)
    st.mask = const.tile([128, 128], BF)
    nc.scalar.copy(st.mask[:], maskf[:])
    identf = const.tile([128, 128], F32)
    make_identity(nc, identf[:])
    st.ident = const.tile([128, 128], BF)
    nc.scalar.copy(st.ident[:], identf[:])

    st.qt = persist.tile([H, T], BF)            # Q^T (pre-scaled by 1/sqrt(H))
    st.kt = persist.tile([H, T], BF)            # K^T
    st.vt2 = persist.tile([128, T // (2 * KT), 128], BF)  # V^T pair-packed
    st.vsb = persist.tile([128, T // KT, 128], BF)  # natural V tiles, padded
    # vsb column layout: [ones | zero pad x63 | V x64] so the O matmul puts
    # the softmax denominator on PSUM partition 0 and O^T on 64:128
    nc.vector.memset(st.vsb[:, :, 0:1], 1.0)
    nc.vector.memset(st.vsb[:, :, 1:H], 0.0)

    # preload the ACT exp table (~1.3us) during the DMA-wait head
    dummy = const.tile([1, 2], F32)
    nc.scalar.activation(dummy[:, 0:1], maskf[0:1, 0:1], EXP)

    # PE warmup: ~4us of back-to-back dummy matmuls while the first DMAs
    # land, so the tensor engine reaches its max p-state before real work
    warm = st.ps_s.tile([128, 2, GQ], F32, name="s_ps")
    for _ in range(36):
        nc.tensor.matmul(warm[:, 0, 0:128], st.ident[:], st.ident[:],
                         start=True, stop=True)

    # ---- flat cross-group software-pipelined schedule ----
    # The O matmuls run one pair behind the scores matmuls, ACROSS group
    # boundaries, so the PE always has independent work queued while an exp
    # completes. Next-group projection chunks ride the front half of each
    # group's pairs (their PSUM->SBUF copies land early in the ACT/DVE
    # queues); V transposes ride pair 1 of their own group.
    _proj_prologue(st)
    _proj_tr(st, 0)
    chunk_at, copies_at, tr_at = {}, {}, {}
    for g in range(NG - 1):
        front = max(1, (2 * g + 2) // 2)
        for i in range(CC):
            chunk_at.setdefault((g, i * front // CC), []).append(i)
        copies_at[(g, front - 1)] = g + 1
        # transposes ride the LAST pair of group g: vtr(g+1) accumulation and
        # copies are done by then, and vtr(g+2) isn't created until (g+1, 0)
        tr_at[(g, 2 * g + 1)] = g + 1
    o_ps_of = {}
    prev = None
    for g in range(NG):
        sl = slice(GQ * g, GQ * (g + 1))
        for p in range(2 * g + 2):
            if p == 0:
                o_ps_of[g] = (st.ps_o.tile([128, GQ], F32, name="o_ps"),)
            cur = (g, p, sl, _pair_scores(st, g, p, sl))
            if prev is not None:
                pg, pp, psl, ppt = prev
                _pair_finish(st, pg, pp, psl, ppt, o_ps_of[pg])
                if pp == 2 * pg + 1:
                    _normalize(st, pg, psl, o_ps_of[pg][0], 0, GQ)
            _proj_chunks(st, g + 1, chunk_at.get((g, p), []))
            if (g, p) in copies_at:
                _proj_copies(st, copies_at[(g, p)])
            if (g, p) in tr_at:
                _proj_tr(st, tr_at[(g, p)])
            prev = cur
    pg, pp, psl, ppt = prev
    _pair_finish(st, pg, pp, psl, ppt, o_ps_of[pg])
    _normalize(st, pg, psl, o_ps_of[pg][0], 0, GQ)


def xT_first(st, lo, hi):
    return st.xT[128 * lo : 128 * hi, 0:GQ].rearrange("(a p) t -> p a t", p=128)


def _proj_prologue(st):
    """Group 0 projection (x0 DMAs already issued up top): all qk matmuls
    before the v matmuls so the Q/K copy-out (which gates the first scores
    matmul) overlaps the v matmuls on the PE."""
    nc = st.nc
    st.qk_ps = st.ps_qk.tile([128, GQ], F32)
    st.vtr = st.ps_vtr.tile([128, GQ], F32)
    for ci in range(CC):
        nc.tensor.matmul(st.qk_ps[:], st.wsb[:, ci, 0:128], st.x0[:, ci, :],
                         start=(ci == 0), stop=(ci == CC - 1))
    _proj_copies_qk(st, 0)
    for ci in range(CC):
        nc.tensor.matmul(st.vtr[0:H, :], st.wsb[:, ci, 128:192],
                         st.x0[:, ci, :],
                         start=(ci == 0), stop=(ci == CC - 1))
    _proj_copies_v(st, 0)


def _proj_chunks(st, g, cis):
    nc = st.nc
    sl = slice(GQ * g, GQ * (g + 1))
    for ci in cis:
        if ci == 0:
            st.qk_ps = st.ps_qk.tile([128, GQ], F32)
            st.vtr = st.ps_vtr.tile([128, GQ], F32)
        if ci % 4 == 0:
            # one batched DMA per 4 C-chunks (4 KB/partition lines)
            st.xt_t = st.xt_pool.tile([128, 4, GQ], BF)
            nc.sync.dma_start(
                out=st.xt_t[:],
                in_=st.xT[512 * (ci // 4) : 512 * (ci // 4 + 1), sl].rearrange(
                    "(a p) t -> p a t", p=128))
        xt = st.xt_t[:, ci % 4, :]
        nc.tensor.matmul(st.qk_ps[:], st.wsb[:, ci, 0:128], xt,
                         start=(ci == 0), stop=(ci == CC - 1))
        nc.tensor.matmul(st.vtr[0:H, :], st.wsb[:, ci, 128:192], xt,
                         start=(ci == 0), stop=(ci == CC - 1))


def _proj_copies_qk(st, g):
    nc = st.nc
    sl = slice(GQ * g, GQ * (g + 1))
    nc.vector.tensor_copy(st.kt[:, sl], st.qk_ps[H:128, :])  # shifted, DVE
    nc.vector.tensor_copy(st.qt[:, sl], st.qk_ps[0:H, :])


def _proj_copies_v(st, g):
    # V^T -> pair-packed vt2
    nc = st.nc
    for e in range(2):
        u = 2 * g + e
        nc.vector.tensor_copy(st.vt2[0:H, u, :],
                              st.vtr[0:H, 256 * e : 256 * e + 128])
        nc.scalar.copy(st.vt2[H:128, u, :],      # shifted -> ACT
                       st.vtr[0:H, 256 * e + 128 : 256 * e + 256])


def _proj_copies(st, g):
    _proj_copies_qk(st, g)
    _proj_copies_v(st, g)


def _proj_tr(st, g):
    # PE-transpose vt2 pairs to natural V tiles (fills the group-end drain)
    nc = st.nc
    vtr_b = st.vtr.bitcast(BF)                  # reuse the V PSUM bank
    for e in range(2):
        u = 2 * g + e
        nc.tensor.transpose(vtr_b[:, 128 * e : 128 * (e + 1)],
                            st.vt2[:, u, :], st.ident[:])
        nc.vector.tensor_copy(st.vsb[:, 2 * u, H:128],
                              vtr_b[:, 128 * e : 128 * e + H])
        nc.vector.tensor_copy(st.vsb[:, 2 * u + 1, H:128],
                              vtr_b[:, 128 * e + H : 128 * e + 128])


def _pair_scores(st, g, p, sl):
    # columns below the pair's min qlo are fully masked for BOTH tiles of
    # the pair -- skip them in the scores matmuls and the exp
    nc = st.nc
    lo = max(0, 128 * (2 * p - 4 * g))
    s_ps = st.ps_s.tile([128, 2, GQ], F32)
    for e in range(2):
        j = 2 * p + e
        nc.tensor.matmul(s_ps[:, e, lo:GQ], st.kt[:, KT * j : KT * (j + 1)],
                         st.qt[:, GQ * g + lo : GQ * (g + 1)],
                         start=True, stop=True)
    pt_t = st.pt_pool.tile([128, 2, GQ], BF)
    nc.scalar.activation(pt_t[:, :, lo:GQ], s_ps[:, :, lo:GQ], EXP)
    return pt_t


def _pair_finish(st, g, p, sl, pt_t, o_ps):
    # o_ps: tuple of 1 (single-bank) or 2 (column-split, last group) tiles
    nc = st.nc
    jmax = 4 * g + 3
    for e in range(2):
        j = 2 * p + e
        s = j - 4 * g                           # diagonal sub-block index
        if s >= 0:
            qlo = 128 * s
            nc.vector.tensor_mul(pt_t[:, e, qlo : qlo + 128],
                                 pt_t[:, e, qlo : qlo + 128], st.mask[:])
    for e in range(2):
        j = 2 * p + e
        qlo = max(0, 128 * (j - 4 * g))
        if len(o_ps) == 1:
            nc.tensor.matmul(o_ps[0][:, qlo:GQ], st.vsb[:, j, :],
                             pt_t[:, e, qlo:GQ],
                             start=(j == 0), stop=(j == jmax))
        else:
            oa, ob = o_ps
            if qlo < 256:
                nc.tensor.matmul(oa[:, qlo:256], st.vsb[:, j, :],
                                 pt_t[:, e, qlo:256],
                                 start=(j == 0), stop=(j == 4 * g + 1))
            lob = max(qlo, 256)
            nc.tensor.matmul(ob[:, lob:GQ], st.vsb[:, j, :],
                             pt_t[:, e, lob:GQ],
                             start=(j == 0), stop=(j == jmax))


def _normalize(st, g, sl, o_ps, lo, hi):
    nc = st.nc
    rec = st.div_pool.tile([1, GQ], F32, name="rec")
    nc.vector.reciprocal_approx_fast(rec[:, lo:hi], o_ps[0:1, lo:hi])
    dbc = st.div_pool.tile([128, GQ], F32, name="dbc")
    nc.gpsimd.partition_broadcast(dbc[:, lo:hi], rec[:, lo:hi])
    osb = st.out_pool.tile([128, GQ], F32, name="osb")
    nc.vector.tensor_mul(osb[H:128, lo:hi], o_ps[H:128, lo:hi],
                         dbc[H:128, lo:hi])
    nc.sync.dma_start(out=st.outT[:, GQ * g + lo : GQ * g + hi],
                      in_=osb[H:128, lo:hi])


def build():
    nc = bacc.Bacc("TRN2", target_bir_lowering=False, debug=False)
    with tile.TileContext(nc) as tc:
        with ExitStack() as ctx:
            _emit(ctx, tc)
    nc.compile()
    return nc


_NC_CACHE = None


def _get_module():
    global _NC_CACHE
    if _NC_CACHE is None:
        _NC_CACHE = build()
    return _NC_CACHE


def prep_in_maps(x, Wq, Wk, Wv):
    x = np.asarray(x, dtype=np.float32)
    Wq = np.asarray(Wq, dtype=np.float32)
    Wk = np.asarray(Wk, dtype=np.float32)
    Wv = np.asarray(Wv, dtype=np.float32)
    bf16 = ml_dtypes.bfloat16
    # [C, 192] = [Wq/sqrt(H) | Wk | Wv], tiled to [128, CC, 192]
    wcat = np.concatenate([Wq * (1.0 / np.sqrt(H)), Wk, Wv], axis=1)
    wqkv = np.ascontiguousarray(
        wcat.reshape(CC, 128, 3 * H).transpose(1, 0, 2).astype(bf16))
    return [
        {"xT": np.ascontiguousarray(x[b].T.astype(bf16)), "wqkv": wqkv}
        for b in range(B)
    ]


def assemble_out(results):
    out = np.empty((B, T, H), dtype=np.float32)
    for b in range(B):
        out[b] = results[b]["outT"].T
    return out


def run(x, Wq, Wk, Wv, trace=False):
    nc = _get_module()
    in_maps = prep_in_maps(x, Wq, Wk, Wv)
    res = run_bass_kernel_spmd(nc, in_maps, core_ids=list(range(N_CORES)),
                               trace=trace)
    return assemble_out(res.results), res


def kernel(x, Wq, Wk, Wv):
    out, _ = run(x, Wq, Wk, Wv)
    return out


# revision 45
# speedup vs baseline: 1.4437x; 1.0090x over previous
"""Single-head causal attention (B=8, T=2048, C=1024, H=64) on 8 TRN2 NeuronCores.

Sharding: data-parallel over batch -- core b computes batch element b. No
collectives. Per core, for x_b [T, C]:
    q = x_b @ Wq / sqrt(H); k = x_b @ Wk; v = x_b @ Wv
    out = softmax(causal(q @ k.T)) @ v

v3 layout strategy (all PE work in bf16, 1 cycle/row + FWL weight loads):
  - Host passes xT = x_b.T [C, T] pre-converted to bf16 (halves input DMA)
    and wqkv packed [128, 8, 192] bf16 (Wq/sqrt(H) | Wk | Wv per C-chunk) so
    weights land in one contiguous DMA (chunk 0 shipped first).
  - Projections per 512-wide t-group: accumulate QK^T [128, 512] and V^T
    [64, 512] in PSUM over 8 C-chunks. Q^T and K^T are copied out REPLICATED
    on both partition halves (qq/kk [128, T]) so the two scores matmuls of a
    k-tile pair run CONCURRENTLY in disjoint 64-row groups of the PE array
    (contraction is only 64 deep). V^T is re-tiled pairwise into vt2
    [128, 8, 128]; ONE PE transpose per pair of 128-k-tiles yields natural V
    [128k, 2x64h]. V tiles live padded in vsb [128, 16, 128]: col 64 = ones
    (softmax denominators fall out of the O matmul), cols 65:128 = zero.
  - Attention per 512-wide q-group g over k-tile pairs: two concurrent
    full-width scores matmuls S^T = K_j^T.T @ Q^T into a 2-bank PSUM tile,
    ONE exp over [128, 1024] on ACT (no max subtraction; scores bounded
    ~+-7), causal 0/1 mask multiply on diagonal 128x128 sub-blocks, then
    per-tile O^T[128, qlo:512] += Vpad_j.T @ P^T accumulated in PSUM (row 64
    = softmax denominator). The O matmuls of a pair are issued one pair
    BEHIND the scores matmuls, and the next group's projection matmuls are
    interleaved into the attention stream, so the PE never drains waiting
    for exp.
  - Normalize: ACT copy of the denominator row to partition 0, DVE
    reciprocal_approx_fast, GPSIMD partition broadcast, DVE multiply; DMA
    O^T [64, T] f32 out. Host transposes back.
"""

from contextlib import ExitStack

import ml_dtypes
import numpy as np

import concourse.mybir as mybir
import concourse.tile as tile
from concourse import bacc
from concourse.bass_utils import run_bass_kernel_spmd
from concourse.masks import make_identity, make_upper_triangular

B, T, C, H = 8, 2048, 1024, 64
N_CORES = 8
GQ = 512          # q-group width (PSUM bank)
NG = T // GQ      # 4 q-groups
KT = 128          # k-tile size
CC = C // 128     # 8 contraction chunks
F32 = mybir.dt.float32
BF = mybir.dt.bfloat16
EXP = mybir.ActivationFunctionType.Exp


class _State:
    pass


def _emit(ctx, tc):
    nc = tc.nc
    st = _State()
    st.nc = nc
    st.xT = nc.dram_tensor("xT", [C, T], BF, kind="ExternalInput").ap()
    wqkv = nc.dram_tensor("wqkv", [128, CC, 3 * H], BF, kind="ExternalInput").ap()
    st.outT = nc.dram_tensor("outT", [H, T], F32, kind="ExternalOutput").ap()

    const = ctx.enter_context(tc.tile_pool(name="const", bufs=1))
    persist = ctx.enter_context(tc.tile_pool(name="persist", bufs=1))
    st.xt_pool = ctx.enter_context(tc.tile_pool(name="xt", bufs=4))
    st.pt_pool = ctx.enter_context(tc.tile_pool(name="pt", bufs=4))
    st.out_pool = ctx.enter_context(tc.tile_pool(name="outp", bufs=2))
    st.div_pool = ctx.enter_context(tc.tile_pool(name="div", bufs=2))
    st.ps_s = ctx.enter_context(tc.tile_pool(name="ps_s", bufs=2, space="PSUM"))
    st.ps_o = ctx.enter_context(tc.tile_pool(name="ps_o", bufs=2, space="PSUM"))
    st.ps_qk = ctx.enter_context(tc.tile_pool(name="ps_qk", bufs=1, space="PSUM"))
    st.ps_vtr = ctx.enter_context(tc.tile_pool(name="ps_vtr", bufs=1, space="PSUM"))

    st.wsb = const.tile([128, CC, 3 * H], BF)
    st.x0 = persist.tile([128, CC, GQ], BF)     # prologue x tiles (group 0)
    # first compute needs x0 chunk 0 + wsb chunk 0: trigger those DMAs first
    nc.sync.dma_start(out=st.x0[:, 0:1, :],
                      in_=xT_first(st, 0, 1))
    nc.sync.dma_start(out=st.wsb[:, 0:2, :], in_=wqkv[:, 0:2, :])
    nc.sync.dma_start(out=st.x0[:, 1:4, :],
                      in_=xT_first(st, 1, 4))
    nc.sync.dma_start(out=st.wsb[:, 2:CC, :], in_=wqkv[:, 2:CC, :])
    nc.sync.dma_start(out=st.x0[:, 4:CC, :],
                      in_=xT_first(st, 4, CC))
    # mask[p, f] = 1.0 where p <= f else 0 : keep k_local <= q_local.
    maskf = const.tile([128, 128], F32)
    make_upper_triangular(nc, maskf[:], val=1.0, diag=True)
    st.mask = const.tile([128, 128], BF)
    nc.scalar.copy(st.mask[:], maskf[:])
    identf = const.tile([128, 128], F32)
    make_identity(nc, identf[:])
    st.ident = const.tile([128, 128], BF)
    nc.scalar.copy(st.ident[:], identf[:])

    st.qt = persist.tile([H, T], BF)            # Q^T (pre-scaled by 1/sqrt(H))
    st.kt = persist.tile([H, T], BF)            # K^T
    st.vt2 = persist.tile([128, T // (2 * KT), 128], BF)  # V^T pair-packed
    st.vsb = persist.tile([128, T // KT, 128], BF)  # natural V tiles, padded
    # vsb column layout: [ones | zero pad x63 | V x64] so the O matmul puts
    # the softmax denominator on PSUM partition 0 and O^T on 64:128
    nc.vector.memset(st.vsb[:, :, 0:1], 1.0)
    nc.vector.memset(st.vsb[:, :, 1:H], 0.0)

    # preload the ACT exp table (~1.3us) during the DMA-wait head
    dummy = const.tile([1, 2], F32)
    nc.scalar.activation(dummy[:, 0:1], maskf[0:1, 0:1], EXP)

    # PE warmup: ~4us of back-to-back dummy matmuls while the first DMAs
    # land, so the tensor engine reaches its max p-state before real work
    warm = st.ps_s.tile([128, 2, GQ], F32, name="s_ps")
    for _ in range(36):
        nc.tensor.matmul(warm[:, 0, 0:128], st.ident[:], st.ident[:],
                         start=True, stop=True)

    # ---- flat cross-group software-pipelined schedule ----
    # The O matmuls run one pair behind the scores matmuls, ACROSS group
    # boundaries, so the PE always has independent work queued while an exp
    # completes. Next-group projection chunks ride the front half of each
    # group's pairs (their PSUM->SBUF copies land early in the ACT/DVE
    # queues); V transposes ride pair 1 of their own group.
    _proj_prologue(st)
    _proj_tr(st, 0)
    chunk_at, copies_at, tr_at = {}, {}, {}
    for g in range(NG - 1):
        front = max(1, (2 * g + 2) // 2)
        for i in range(CC):
            chunk_at.setdefault((g, i * front // CC), []).append(i)
        copies_at[(g, front - 1)] = g + 1
        # transposes ride the LAST pair of group g: vtr(g+1) accumulation and
        # copies are done by then, and vtr(g+2) isn't created until (g+1, 0)
        tr_at[(g, 2 * g + 1)] = g + 1
    o_ps_of = {}
    prev = None
    for g in range(NG):
        sl = slice(GQ * g, GQ * (g + 1))
        for p in range(2 * g + 2):
            if p == 0:
                o_ps_of[g] = (st.ps_o.tile([128, GQ], F32, name="o_ps"),)
            cur = (g, p, sl, _pair_scores(st, g, p, sl))
            if prev is not None:
                pg, pp, psl, ppt = prev
                _pair_finish(st, pg, pp, psl, ppt, o_ps_of[pg])
                if pp == 2 * pg + 1:
                    _normalize(st, pg, psl, o_ps_of[pg][0], 0, GQ)
            _proj_chunks(st, g + 1, chunk_at.get((g, p), []))
            if (g, p) in copies_at:
                _proj_copies(st, copies_at[(g, p)])
            if (g, p) in tr_at:
                _proj_tr(st, tr_at[(g, p)])
            prev = cur
    pg, pp, psl, ppt = prev
    _pair_finish(st, pg, pp, psl, ppt, o_ps_of[pg])
    _normalize(st, pg, psl, o_ps_of[pg][0], 0, GQ)


def xT_first(st, lo, hi):
    return st.xT[128 * lo : 128 * hi, 0:GQ].rearrange("(a p) t -> p a t", p=128)


def _proj_prologue(st):
    """Group 0 projection (x0 DMAs already issued up top): all qk matmuls
    before the v matmuls so the Q/K copy-out (which gates the first scores
    matmul) overlaps the v matmuls on the PE."""
    nc = st.nc
    st.qk_ps = st.ps_qk.tile([128, GQ], F32)
    st.vtr = st.ps_vtr.tile([128, GQ], F32)
    for ci in range(CC):
        nc.tensor.matmul(st.qk_ps[:], st.wsb[:, ci, 0:128], st.x0[:, ci, :],
                         start=(ci == 0), stop=(ci == CC - 1))
    _proj_copies_qk(st, 0)
    for ci in range(CC):
        nc.tensor.matmul(st.vtr[0:H, :], st.wsb[:, ci, 128:192],
                         st.x0[:, ci, :],
                         start=(ci == 0), stop=(ci == CC - 1))
    _proj_copies_v(st, 0)


def _proj_chunks(st, g, cis):
    nc = st.nc
    sl = slice(GQ * g, GQ * (g + 1))
    for ci in cis:
        if ci == 0:
            st.qk_ps = st.ps_qk.tile([128, GQ], F32)
            st.vtr = st.ps_vtr.tile([128, GQ], F32)
        if ci % 4 == 0:
            # one batched DMA per 4 C-chunks (4 KB/partition lines)
            st.xt_t = st.xt_pool.tile([128, 4, GQ], BF)
            nc.sync.dma_start(
                out=st.xt_t[:],
                in_=st.xT[512 * (ci // 4) : 512 * (ci // 4 + 1), sl].rearrange(
                    "(a p) t -> p a t", p=128))
        xt = st.xt_t[:, ci % 4, :]
        nc.tensor.matmul(st.qk_ps[:], st.wsb[:, ci, 0:128], xt,
                         start=(ci == 0), stop=(ci == CC - 1))
        nc.tensor.matmul(st.vtr[0:H, :], st.wsb[:, ci, 128:192], xt,
                         start=(ci == 0), stop=(ci == CC - 1))


def _proj_copies_qk(st, g):
    nc = st.nc
    sl = slice(GQ * g, GQ * (g + 1))
    nc.vector.tensor_copy(st.kt[:, sl], st.qk_ps[H:128, :])  # shifted, DVE
    nc.vector.tensor_copy(st.qt[:, sl], st.qk_ps[0:H, :])


def _proj_copies_v(st, g):
    # V^T -> pair-packed vt2
    nc = st.nc
    for e in range(2):
        u = 2 * g + e
        nc.vector.tensor_copy(st.vt2[0:H, u, :],
                              st.vtr[0:H, 256 * e : 256 * e + 128])
        nc.vector.tensor_copy(st.vt2[H:128, u, :],   # shifted, DVE
                              st.vtr[0:H, 256 * e + 128 : 256 * e + 256])


def _proj_copies(st, g):
    _proj_copies_qk(st, g)
    _proj_copies_v(st, g)


def _proj_tr(st, g):
    # PE-transpose vt2 pairs to natural V tiles (fills the group-end drain)
    nc = st.nc
    vtr_b = st.vtr.bitcast(BF)                  # reuse the V PSUM bank
    for e in range(2):
        u = 2 * g + e
        nc.tensor.transpose(vtr_b[:, 128 * e : 128 * (e + 1)],
                            st.vt2[:, u, :], st.ident[:])
        nc.vector.tensor_copy(st.vsb[:, 2 * u, H:128],
                              vtr_b[:, 128 * e : 128 * e + H])
        nc.vector.tensor_copy(st.vsb[:, 2 * u + 1, H:128],
                              vtr_b[:, 128 * e + H : 128 * e + 128])


def _pair_scores(st, g, p, sl):
    # columns below the pair's min qlo are fully masked for BOTH tiles of
    # the pair -- skip them in the scores matmuls and the exp
    nc = st.nc
    lo = max(0, 128 * (2 * p - 4 * g))
    s_ps = st.ps_s.tile([128, 2, GQ], F32)
    for e in range(2):
        j = 2 * p + e
        nc.tensor.matmul(s_ps[:, e, lo:GQ], st.kt[:, KT * j : KT * (j + 1)],
                         st.qt[:, GQ * g + lo : GQ * (g + 1)],
                         start=True, stop=True)
    pt_t = st.pt_pool.tile([128, 2, GQ], BF)
    nc.scalar.activation(pt_t[:, :, lo:GQ], s_ps[:, :, lo:GQ], EXP)
    return pt_t


def _pair_finish(st, g, p, sl, pt_t, o_ps):
    # o_ps: tuple of 1 (single-bank) or 2 (column-split, last group) tiles
    nc = st.nc
    jmax = 4 * g + 3
    for e in range(2):
        j = 2 * p + e
        s = j - 4 * g                           # diagonal sub-block index
        if s >= 0:
            qlo = 128 * s
            nc.vector.tensor_mul(pt_t[:, e, qlo : qlo + 128],
                                 pt_t[:, e, qlo : qlo + 128], st.mask[:])
    for e in range(2):
        j = 2 * p + e
        qlo = max(0, 128 * (j - 4 * g))
        if len(o_ps) == 1:
            nc.tensor.matmul(o_ps[0][:, qlo:GQ], st.vsb[:, j, :],
                             pt_t[:, e, qlo:GQ],
                             start=(j == 0), stop=(j == jmax))
        else:
            oa, ob = o_ps
            if qlo < 256:
                nc.tensor.matmul(oa[:, qlo:256], st.vsb[:, j, :],
                                 pt_t[:, e, qlo:256],
                                 start=(j == 0), stop=(j == 4 * g + 1))
            lob = max(qlo, 256)
            nc.tensor.matmul(ob[:, lob:GQ], st.vsb[:, j, :],
                             pt_t[:, e, lob:GQ],
                             start=(j == 0), stop=(j == jmax))


def _normalize(st, g, sl, o_ps, lo, hi):
    nc = st.nc
    rec = st.div_pool.tile([1, GQ], F32, name="rec")
    nc.vector.reciprocal_approx_fast(rec[:, lo:hi], o_ps[0:1, lo:hi])
    dbc = st.div_pool.tile([128, GQ], F32, name="dbc")
    nc.gpsimd.partition_broadcast(dbc[:, lo:hi], rec[:, lo:hi])
    osb = st.out_pool.tile([128, GQ], F32, name="osb")
    nc.vector.tensor_mul(osb[H:128, lo:hi], o_ps[H:128, lo:hi],
                         dbc[H:128, lo:hi])
    nc.sync.dma_start(out=st.outT[:, GQ * g + lo : GQ * g + hi],
                      in_=osb[H:128, lo:hi])


def build():
    nc = bacc.Bacc("TRN2", target_bir_lowering=False, debug=False)
    with tile.TileContext(nc) as tc:
        with ExitStack() as ctx:
            _emit(ctx, tc)
    nc.compile()
    return nc


_NC_CACHE = None


def _get_module():
    global _NC_CACHE
    if _NC_CACHE is None:
        _NC_CACHE = build()
    return _NC_CACHE


def prep_in_maps(x, Wq, Wk, Wv):
    x = np.asarray(x, dtype=np.float32)
    Wq = np.asarray(Wq, dtype=np.float32)
    Wk = np.asarray(Wk, dtype=np.float32)
    Wv = np.asarray(Wv, dtype=np.float32)
    bf16 = ml_dtypes.bfloat16
    # [C, 192] = [Wq/sqrt(H) | Wk | Wv], tiled to [128, CC, 192]
    wcat = np.concatenate([Wq * (1.0 / np.sqrt(H)), Wk, Wv], axis=1)
    wqkv = np.ascontiguousarray(
        wcat.reshape(CC, 128, 3 * H).transpose(1, 0, 2).astype(bf16))
    return [
        {"xT": np.ascontiguousarray(x[b].T.astype(bf16)), "wqkv": wqkv}
        for b in range(B)
    ]


def assemble_out(results):
    out = np.empty((B, T, H), dtype=np.float32)
    for b in range(B):
        out[b] = results[b]["outT"].T
    return out


def run(x, Wq, Wk, Wv, trace=False):
    nc = _get_module()
    in_maps = prep_in_maps(x, Wq, Wk, Wv)
    res = run_bass_kernel_spmd(nc, in_maps, core_ids=list(range(N_CORES)),
                               trace=trace)
    return assemble_out(res.results), res


def kernel(x, Wq, Wk, Wv):
    out, _ = run(x, Wq, Wk, Wv)
    return out
